# revision 38
# baseline (speedup 1.0000x reference)
"""Trainium2 Bass kernel for nn_M04AdaptiveVQ.

Data-parallel over B: each of the 8 NeuronCores processes one batch element
(1500 frames). Centroid bank, MLP weights and RVQ codebooks are replicated.

Per-core pipeline (all layouts feature-major (feature, frame) unless noted):
  1. VQ: score[t,k] = 2*x_t.c_k + (C - |c_k|^2) via bf16 matmuls (8 k-chunks
     of 1024, streamed), DVE add (f32) + max8/max_index per chunk, 64-way
     candidate combine -> global top-2 per frame.
  2. Exact fp32 rescore of the two candidates per frame (PE dot trick),
     then gather winner centroid rows (SWDGE) -> PE-transpose to (C,T).
  3. spk_raw = feature - lin_dec; enc MLP bf16 (Lrelu on ACT).
  4. nrm MLP bf16 (Relu); spk_enc_norm = spk_enc / (norm+eps).
  5. RVQ: 4 stages; bf16 distance matmuls + bf16 scans; codebook row
     "gather" via one-hot matmul (no DMA in the dependency chain);
     residual kept in f32 master.
  6. dec MLP bf16; out = lin_dec + spk_dec.
"""
import sys
if '/opt/trn_rl_repo' not in sys.path:
    sys.path.insert(0, '/opt/trn_rl_repo')
import numpy as np

B, C, T = 8, 1024, 1500
K = 8192
D, H = 256, 512
NQ, BINS = 4, 1024
TPAD = 1536
NFT = TPAD // 128      # 12 frame tiles
NKC = K // 1024        # 8 centroid chunks (1024 wide)
NCC = C // 128         # 8
NDC = D // 128         # 2
NHC = H // 128         # 4
NTT = TPAD // 512      # 3 mlp t-tiles
EPS = 1e-8
CSHIFT = float(C)
CBSHIFT = float(D)

TRACE = False
LAST_RESULT = None
_prog = None


def _build_program():
    import concourse.bacc as bacc
    import concourse.mybir as mybir
    from concourse.tile import TileContext
    from concourse.masks import make_identity
    from contextlib import ExitStack

    f32 = mybir.dt.float32
    bf16 = mybir.dt.bfloat16
    i16 = mybir.dt.int16
    u16 = mybir.dt.uint16
    AF = mybir.ActivationFunctionType
    OP = mybir.AluOpType
    AX = mybir.AxisListType

    nc = bacc.Bacc("TRN2", target_bir_lowering=False, debug=False, num_devices=8)

    dfeatbf = nc.dram_tensor("featbf", [C, TPAD], bf16, kind="ExternalInput")
    dfeat = nc.dram_tensor("feat", [C, TPAD], f32, kind="ExternalInput")
    dcentbf = nc.dram_tensor("centbf", [C, K], bf16, kind="ExternalInput")
    dcnc = nc.dram_tensor("cnc", [128, K], f32, kind="ExternalInput")
    dcent = nc.dram_tensor("cent", [K, C], f32, kind="ExternalInput")
    dcent_ext = nc.dram_tensor("cent_ext", [K, 1088], f32, kind="ExternalInput")
    dbase64 = nc.dram_tensor("base64", [128, 64], f32, kind="ExternalInput")
    dbinbase = nc.dram_tensor("binbase", [128, 8], f32, kind="ExternalInput")
    dw = {}
    for pre, shapes in (
        ("enc", [(C, H), (H, H), (H, D)]),
        ("nrm", [(C, H), (H, H), (H, D)]),
        ("dec", [(D, H), (H, H), (H, C)]),
    ):
        for i, (ni, no) in enumerate(shapes):
            dw[f"{pre}_w{i}"] = nc.dram_tensor(f"{pre}_w{i}", [ni, no], bf16,
                                               kind="ExternalInput")
            dw[f"{pre}_b{i}"] = nc.dram_tensor(f"{pre}_b{i}", [no], f32,
                                               kind="ExternalInput")
    dcbT2 = nc.dram_tensor("cbT2bf", [NQ, D, BINS], bf16, kind="ExternalInput")
    dcbnc = nc.dram_tensor("cbnc", [NQ, BINS], f32, kind="ExternalInput")
    dcbmm = nc.dram_tensor("cbmm", [128, NQ, 8, NDC, 128], bf16,
                           kind="ExternalInput")
    dout = nc.dram_tensor("out", [C, T], f32, kind="ExternalOutput")

    def wrap_idx(gidxs, idxw, eng=None):
        # gidxs (128, NFT) u16: [p, j] = idx of frame j*128+p
        # idxw (128, 96) i16: [q, 8a+b] = idx of frame a*128 + 16b+q
        eng = eng or nc.sync
        idxw3 = idxw[:].rearrange("p (a b) -> p a b", b=8)
        for b in range(8):
            eng.dma_start(out=idxw3[0:16, :, b],
                          in_=gidxs[16 * b:16 * b + 16, :].bitcast(i16))
        for g in range(1, 8):
            eng.dma_start(out=idxw[16 * g:16 * g + 16, :], in_=idxw[0:16, :])

    def load_w(pool, pre, i, ni, no):
        a, b = ni // 128, no // 128
        wt = pool.tile([128, a, b, 128], bf16, tag=f"{pre}_w{i}")
        nc.sync.dma_start(
            out=wt[:],
            in_=dw[f"{pre}_w{i}"][:].rearrange("(a p) (b q) -> p a b q",
                                               p=128, q=128))
        bt = pool.tile([128, b], f32, tag=f"{pre}_b{i}")
        nc.sync.dma_start(out=bt[:],
                          in_=dw[f"{pre}_b{i}"][:].rearrange("(a p) -> p a", p=128))
        return wt, bt

    def mlp_layer(mp, wb, in_t, out_t, ic, oc, func, alpha=0.0):
        wt, bt = wb
        for hc in range(oc):
            ps = mp.tile([128, 512], f32, tag="mlp_ps")
            for cc in range(ic):
                nc.tensor.matmul(ps[:], wt[:, cc, hc], in_t[:, cc],
                                 start=(cc == 0), stop=(cc == ic - 1))
            nc.scalar.activation(out_t[:, hc], ps[:], func,
                                 bias=bt[:, hc:hc + 1], scale=1.0, alpha=alpha)

    with TileContext(nc) as tc:
        with ExitStack() as top:
            const = top.enter_context(tc.tile_pool(name="const", bufs=1))
            ident = const.tile([128, 128], f32)
            make_identity(nc, ident[:])
            ones1 = const.tile([1, 128], f32)
            nc.vector.memset(ones1[:], 1.0)
            identb = const.tile([128, 128], bf16)
            nc.vector.tensor_copy(identb[:], ident[:])
            base64_t = const.tile([128, 64], f32)
            nc.sync.dma_start(out=base64_t[:], in_=dbase64[:])
            binbase_t = const.tile([128, 8], f32)
            nc.sync.dma_start(out=binbase_t[:], in_=dbinbase[:])
            gidxs = const.tile([128, NFT], u16)
            idxw = const.tile([128, 96], i16)
            gi1 = const.tile([128, NFT], u16)
            gi2 = const.tile([128, NFT], u16)

            # whole-program tensors: lin_dec, dec weights, sed
            wlong = top.enter_context(tc.tile_pool(name="wlong", bufs=1))
            ld_t = wlong.tile([128, NCC, TPAD], f32)      # lin_dec, (C, T)
            sed = wlong.tile([128, NDC, TPAD], bf16)      # q_sum * norm_vec

            # ---------------- Phase 1: VQ distance + argmax ----------------
            with tc.tile_pool(name="featp", bufs=1) as featp:
                feat_t = featp.tile([128, NCC, TPAD], bf16)
                for cc in range(NCC):
                    nc.scalar.dma_start(
                        out=feat_t[:, cc],
                        in_=dfeatbf[cc * 128:(cc + 1) * 128, :])
                val64 = featp.tile([128, NFT, 64], f32)
                idx64 = featp.tile([128, NFT, 64], u16)
                with tc.tile_pool(name="cw", bufs=3) as cw, \
                     tc.tile_pool(name="vqps", bufs=3, space="PSUM") as vqps, \
                     tc.tile_pool(name="vqst", bufs=4) as vqst, \
                     tc.tile_pool(name="comb", bufs=2) as comb:
                    for kc in range(NKC):
                        ks = slice(kc * 1024, (kc + 1) * 1024)
                        cwt = cw.tile([128, NCC, 1024], bf16, tag="cw")
                        if kc == 0:
                            for cc in range(NCC):
                                nc.sync.dma_start(
                                    out=cwt[:, cc],
                                    in_=dcentbf[cc * 128:(cc + 1) * 128, ks])
                        else:
                            nc.sync.dma_start(
                                out=cwt[:],
                                in_=dcentbf[:, ks].rearrange(
                                    "(a p) k -> p a k", p=128))
                        cncc = cw.tile([128, 1024], f32, tag="cncc")
                        nc.sync.dma_start(out=cncc[:], in_=dcnc[:, ks])
                        if kc == 1:
                            # prefetch dec weights behind chunk 1
                            wd = [load_w(wlong, "dec", i, *s) for i, s in
                                  enumerate([(D, H), (H, H), (H, C)])]
                        for ft in range(NFT):
                            fs = slice(ft * 128, (ft + 1) * 128)
                            ps = vqps.tile([128, 1024], f32, tag="vqps")
                            for half in range(2):
                                hs = slice(half * 512, (half + 1) * 512)
                                for cc in range(NCC):
                                    nc.tensor.matmul(ps[:, hs],
                                                     feat_t[:, cc, fs],
                                                     cwt[:, cc, hs],
                                                     start=(cc == 0),
                                                     stop=(cc == NCC - 1))
                            dist = vqst.tile([128, 1024], f32, tag="dist")
                            nc.vector.tensor_tensor(out=dist[:], in0=ps[:],
                                                    in1=cncc[:], op=OP.add)
                            v8s = val64[:, ft, kc * 8:(kc + 1) * 8]
                            nc.vector.max(out=v8s, in_=dist[:])
                            nc.vector.max_index(
                                out=idx64[:, ft, kc * 8:(kc + 1) * 8],
                                in_max=v8s, in_values=dist[:])
                            if kc == NKC - 1:
                                # combine the 64 candidates -> rank-0/1
                                idxf = comb.tile([128, 64], f32, tag="idxf")
                                nc.vector.tensor_copy(idxf[:], idx64[:, ft])
                                nc.vector.tensor_tensor(out=idxf[:],
                                                        in0=idxf[:],
                                                        in1=base64_t[:],
                                                        op=OP.add)
                                g8 = comb.tile([128, 8], f32, tag="g8")
                                nc.vector.max(out=g8[:], in_=val64[:, ft])
                                for rank, gi in ((0, gi1), (1, gi2)):
                                    msk = comb.tile([128, 64], f32, tag="msk")
                                    nc.vector.tensor_scalar(
                                        msk[:], val64[:, ft],
                                        g8[:, rank:rank + 1],
                                        None, op0=OP.is_equal)
                                    nc.vector.tensor_tensor(out=msk[:],
                                                            in0=msk[:],
                                                            in1=idxf[:],
                                                            op=OP.mult)
                                    gx = comb.tile([128, 1], f32, tag="gx")
                                    nc.vector.reduce_max(gx[:], msk[:],
                                                         axis=AX.X)
                                    nc.vector.tensor_copy(gi[:, ft:ft + 1],
                                                          gx[:])

            # mid-life pools (LIFO: below wenc/wnrm which close earlier)
            mid_s = ExitStack()
            midp = mid_s.enter_context(tc.tile_pool(name="midp", bufs=1))
            spk_enc = midp.tile([128, NDC, TPAD], f32)
            norm_v = midp.tile([128, NDC, TPAD], f32)
            rq_s = ExitStack()
            rqp = rq_s.enter_context(tc.tile_pool(name="rqp", bufs=1))
            r_f = rqp.tile([128, NDC, TPAD], f32)
            r_b = rqp.tile([128, NDC, TPAD], bf16)
            qs = rqp.tile([128, NDC, TPAD], f32)
            cbs_s = ExitStack()
            cbs = cbs_s.enter_context(tc.tile_pool(name="cbs", bufs=2))

            def load_cb_stage(q):
                cbt = cbs.tile([128, NDC, BINS], bf16, tag="cbt")
                nc.sync.dma_start(
                    out=cbt[:],
                    in_=dcbT2[q].rearrange("(a p) n -> p a n", p=128))
                cbm = cbs.tile([128, 8, NDC, 128], bf16, tag="cbm")
                nc.sync.dma_start(out=cbm[:], in_=dcbmm[:, q])
                cbnq = cbs.tile([1, BINS], f32, tag="cbnq")
                nc.sync.dma_start(out=cbnq[:], in_=dcbnc[q:q + 1, :])
                return cbt, cbm, cbnq

            cb_next = load_cb_stage(0)

            # exact fp32 rescore of the two candidates per frame
            with tc.tile_pool(name="rsc", bufs=2) as rsc, \
                 tc.tile_pool(name="rsi", bufs=1) as rsi, \
                 tc.tile_pool(name="rscps", bufs=4, space="PSUM") as rscps:
                idxw1 = rsi.tile([128, 96], i16)
                idxw2 = rsi.tile([128, 96], i16)
                wrap_idx(gi1, idxw1, nc.sync)
                wrap_idx(gi2, idxw2, nc.scalar)
                for ft in range(NFT):
                    fs = slice(ft * 128, (ft + 1) * 128)
                    g1t = rsc.tile([128, 1, 1088], f32, tag="g1t")
                    nc.gpsimd.dma_gather(out_ap=g1t[:], in_ap=dcent_ext[:],
                                         idxs_ap=idxw1[:, 8 * ft:8 * ft + 8],
                                         num_idxs=128, num_idxs_reg=128,
                                         elem_size=1088)
                    g2t = rsc.tile([128, 1, 1088], f32, tag="g2t")
                    nc.gpsimd.dma_gather(out_ap=g2t[:], in_ap=dcent_ext[:],
                                         idxs_ap=idxw2[:, 8 * ft:8 * ft + 8],
                                         num_idxs=128, num_idxs_reg=128,
                                         elem_size=1088)
                    cd = rsc.tile([128, 1024], f32, tag="cd")
                    nc.vector.tensor_tensor(out=cd[:], in0=g1t[:, 0, 0:1024],
                                            in1=g2t[:, 0, 0:1024],
                                            op=OP.subtract)
                    cdT = rsc.tile([128, NCC, 128], f32, tag="cdT")
                    for cc in range(NCC):
                        pt = rscps.tile([128, 128], f32, tag="rscps")
                        nc.tensor.transpose(
                            pt[:], cd[:, cc * 128:(cc + 1) * 128], ident[:])
                        nc.vector.tensor_copy(cdT[:, cc], pt[:])
                    xf = rsc.tile([128, NCC, 128], f32, tag="xf")
                    nc.scalar.dma_start(
                        out=xf[:],
                        in_=dfeat[:, fs].rearrange("(a p) t -> p a t", p=128))
                    ps2 = rscps.tile([128, 128], f32, tag="rscmm")
                    for cc in range(NCC):
                        nc.tensor.matmul(ps2[:], cdT[:, cc], xf[:, cc],
                                         start=(cc == 0), stop=(cc == NCC - 1))
                    dg = rsc.tile([128, 128], f32, tag="dg")
                    nc.vector.tensor_tensor(out=dg[:], in0=ps2[:],
                                            in1=ident[:], op=OP.mult)
                    sdot = rsc.tile([128, 1], f32, tag="sdot")
                    nc.vector.reduce_sum(sdot[:], dg[:], axis=AX.X)
                    delta = rsc.tile([128, 1], f32, tag="delta")
                    nc.vector.tensor_scalar_mul(delta[:], sdot[:], -2.0)
                    nd = rsc.tile([128, 1], f32, tag="nd")
                    nc.vector.tensor_tensor(out=nd[:],
                                            in0=g1t[:, 0, 1024:1025],
                                            in1=g2t[:, 0, 1024:1025],
                                            op=OP.subtract)
                    nc.vector.tensor_tensor(out=delta[:], in0=delta[:],
                                            in1=nd[:], op=OP.add)
                    selm = rsc.tile([128, 1], mybir.dt.uint32, tag="selm")
                    nc.vector.tensor_scalar(selm[:], delta[:], 0.0, None,
                                            op0=OP.is_gt)
                    nc.vector.tensor_copy(gidxs[:, ft:ft + 1],
                                          gi1[:, ft:ft + 1])
                    nc.vector.copy_predicated(gidxs[:, ft:ft + 1], selm[:],
                                              gi2[:, ft:ft + 1])

            wrap_idx(gidxs, idxw)

            # enc/nrm weights prefetch behind the lin_dec gathers
            ws_enc = ExitStack()
            wenc = ws_enc.enter_context(tc.tile_pool(name="wenc", bufs=1))
            ws_nrm = ExitStack()
            wnrm = ws_nrm.enter_context(tc.tile_pool(name="wnrm", bufs=1))
            we = [load_w(wenc, "enc", i, *s) for i, s in
                  enumerate([(C, H), (H, H), (H, D)])]
            wn = [load_w(wnrm, "nrm", i, *s) for i, s in
                  enumerate([(C, H), (H, H), (H, D)])]

            # ---------------- Phase 2: gather + transpose lin_dec ----------
            with tc.tile_pool(name="lg", bufs=2) as lg, \
                 tc.tile_pool(name="trps", bufs=2, space="PSUM") as trps:
                for ft in range(NFT):
                    g = lg.tile([128, 1, 1024], f32, tag="lg")
                    nc.gpsimd.dma_gather(out_ap=g[:], in_ap=dcent[:],
                                         idxs_ap=idxw[:, 8 * ft:8 * ft + 8],
                                         num_idxs=128, num_idxs_reg=128,
                                         elem_size=1024)
                    for cc in range(NCC):
                        pt = trps.tile([128, 128], f32, tag="trps")
                        nc.tensor.transpose(pt[:],
                                            g[:, 0, cc * 128:(cc + 1) * 128],
                                            ident[:])
                        nc.vector.tensor_copy(
                            ld_t[:, cc, ft * 128:(ft + 1) * 128], pt[:])

            # ---------------- Phase 3: spk_raw + enc MLP ----------------
            with tc.tile_pool(name="henc", bufs=1) as hp, \
                 tc.tile_pool(name="ftmp", bufs=2) as ftmp, \
                 tc.tile_pool(name="mlpps", bufs=6, space="PSUM") as mp:
                for tt in range(NTT):
                    ts_ = slice(tt * 512, (tt + 1) * 512)
                    spk_tt = hp.tile([128, NCC, 512], bf16, tag="spk_tt")
                    for cc in range(NCC):
                        fre = ftmp.tile([128, 512], f32, tag="fre")
                        nc.scalar.dma_start(
                            out=fre[:],
                            in_=dfeat[cc * 128:(cc + 1) * 128, ts_])
                        nc.vector.tensor_tensor(out=spk_tt[:, cc],
                                                in0=fre[:],
                                                in1=ld_t[:, cc, ts_],
                                                op=OP.subtract)
                    h0 = hp.tile([128, NHC, 512], bf16, tag="h0")
                    mlp_layer(mp, we[0], spk_tt, h0, NCC, NHC,
                              AF.Lrelu, alpha=0.01)
                    h1 = hp.tile([128, NHC, 512], bf16, tag="h1")
                    mlp_layer(mp, we[1], h0, h1, NHC, NHC,
                              AF.Lrelu, alpha=0.01)
                    mlp_layer(mp, we[2], h1, spk_enc[:, :, ts_],
                              NHC, NDC, AF.Identity)

            # ---------------- Phase 3.5: nrm MLP ----------------
            with tc.tile_pool(name="hnrm", bufs=1) as hp, \
                 tc.tile_pool(name="ldrp", bufs=1) as ldrp, \
                 tc.tile_pool(name="mlpps2", bufs=6, space="PSUM") as mp:
                for tt in range(NTT):
                    ts_ = slice(tt * 512, (tt + 1) * 512)
                    ldr = ldrp.tile([128, NCC, 512], bf16, tag="ldr")
                    nc.vector.tensor_copy(ldr[:], ld_t[:, :, ts_])
                    n0 = hp.tile([128, NHC, 512], bf16, tag="n0")
                    mlp_layer(mp, wn[0], ldr, n0, NCC, NHC, AF.Relu)
                    n1 = hp.tile([128, NHC, 512], bf16, tag="n1")
                    mlp_layer(mp, wn[1], n0, n1, NHC, NHC, AF.Relu)
                    mlp_layer(mp, wn[2], n1, norm_v[:, :, ts_],
                              NHC, NDC, AF.Relu)
            ws_nrm.close()
            ws_enc.close()

            # ---------------- Phase 4: normalize + RVQ ----------------
            with tc.tile_pool(name="tmpp", bufs=1) as tmpp:
                recip = tmpp.tile([128, NDC, TPAD], f32)
                nc.vector.tensor_scalar_add(recip[:], norm_v[:], EPS)
                nc.vector.reciprocal(recip[:], recip[:])
                nc.vector.tensor_tensor(out=r_f[:], in0=spk_enc[:],
                                        in1=recip[:], op=OP.mult)
                nc.vector.tensor_copy(r_b[:], r_f[:])
                nc.vector.memset(qs[:], 0.0)

            with tc.tile_pool(name="rps", bufs=2, space="PSUM") as rps, \
                 tc.tile_pool(name="rqps", bufs=4, space="PSUM") as rqps, \
                 tc.tile_pool(name="rst", bufs=4) as rst, \
                 tc.tile_pool(name="ohp", bufs=3) as ohp:
                for q in range(NQ):
                    cbt, cbm, cbnq = cb_next
                    if q < NQ - 1:
                        cb_next = load_cb_stage(q + 1)
                    for fp in range(NFT // 2):
                        ohT2 = rst.tile([128, 2, BINS], bf16, tag="ohT")
                        for sub in range(2):
                            ft = fp * 2 + sub
                            fs = slice(ft * 128, (ft + 1) * 128)
                            ps = rps.tile([128, BINS], f32, tag="rps")
                            # rank-1 fold of (D - |cb|^2), then the dot terms
                            for half in range(2):
                                hs = slice(half * 512, (half + 1) * 512)
                                nc.tensor.matmul(ps[:, hs], ones1[:],
                                                 cbnq[:, hs],
                                                 start=True, stop=False)
                                for dc in range(NDC):
                                    nc.tensor.matmul(ps[:, hs],
                                                     r_b[:, dc, fs],
                                                     cbt[:, dc, hs],
                                                     start=False,
                                                     stop=(dc == NDC - 1))
                            distf = rst.tile([128, BINS], f32, tag="rdist")
                            nc.scalar.activation(distf[:], ps[:], AF.Identity)
                            vmax = rst.tile([128, 1], f32, tag="vmax")
                            nc.vector.reduce_max(vmax[:], distf[:], axis=AX.X)
                            nc.vector.tensor_scalar(ohT2[:, sub], distf[:],
                                                    vmax[:], None,
                                                    op0=OP.is_equal)
                        # XBAR block transpose: oh2[p, sub*8+c, f]
                        #   = ohT2[f, sub, c*128+p]
                        oh2 = ohp.tile([128, 16, 128], bf16, tag="oh")
                        nc.sync.dma_start(out=oh2[:], in_=ohT2[:],
                                          transpose=True)
                        psq = rqps.tile([128, NDC, 2, 128], f32, tag="rqps")
                        for dc in range(NDC):
                            for c in range(8):
                                nc.tensor.matmul(
                                    psq[:, dc],
                                    cbm[:, c, dc],
                                    oh2[:, c:c + 9:8, :],
                                    start=(c == 0), stop=(c == 7))
                        for sub in range(2):
                            ft = fp * 2 + sub
                            fs = slice(ft * 128, (ft + 1) * 128)
                            for dc in range(NDC):
                                nc.vector.tensor_tensor(
                                    out=qs[:, dc, fs], in0=qs[:, dc, fs],
                                    in1=psq[:, dc, sub], op=OP.add)
                                if q < NQ - 1:
                                    nc.vector.tensor_tensor(
                                        out=r_f[:, dc, fs],
                                        in0=r_f[:, dc, fs],
                                        in1=psq[:, dc, sub],
                                        op=OP.subtract)
                            if q < NQ - 1:
                                nc.vector.tensor_copy(r_b[:, :, fs],
                                                      r_f[:, :, fs])

            # ---------------- Phase 5: dec MLP + final add ----------------
            # (sed = q_sum * norm_vec computed per t-tile so dec tt0 can
            # start while the RVQ tail of later frame-tiles still drains)
            with tc.tile_pool(name="hdec", bufs=1) as hp, \
                 tc.tile_pool(name="outp", bufs=3) as outp, \
                 tc.tile_pool(name="mlpps3", bufs=6, space="PSUM") as mp:
                for tt in range(NTT):
                    ts_ = slice(tt * 512, (tt + 1) * 512)
                    nc.vector.tensor_tensor(out=sed[:, :, ts_],
                                            in0=qs[:, :, ts_],
                                            in1=norm_v[:, :, ts_],
                                            op=OP.mult)
                    d0 = hp.tile([128, NHC, 512], bf16, tag="d0")
                    mlp_layer(mp, wd[0], sed[:, :, ts_], d0,
                              NDC, NHC, AF.Lrelu, alpha=0.01)
                    d1 = hp.tile([128, NHC, 512], bf16, tag="d1")
                    mlp_layer(mp, wd[1], d0, d1, NHC, NHC,
                              AF.Lrelu, alpha=0.01)
                    n = min(512, T - tt * 512)
                    for hc in range(NCC):
                        ps = mp.tile([128, 512], f32, tag="mlp_ps")
                        for cc in range(NHC):
                            nc.tensor.matmul(ps[:], wd[2][0][:, cc, hc],
                                             d1[:, cc], start=(cc == 0),
                                             stop=(cc == NHC - 1))
                        tmpo = outp.tile([128, 512], f32, tag="tmpo")
                        nc.scalar.activation(tmpo[:], ps[:], AF.Identity,
                                             bias=wd[2][1][:, hc:hc + 1],
                                             scale=1.0)
                        nc.vector.tensor_tensor(out=tmpo[:], in0=tmpo[:],
                                                in1=ld_t[:, hc, ts_],
                                                op=OP.add)
                        nc.sync.dma_start(
                            out=dout[:].rearrange("(a p) t -> p a t", p=128)
                                [:, hc, tt * 512:tt * 512 + n],
                            in_=tmpo[:, 0:n])
            cbs_s.close()
            rq_s.close()
            mid_s.close()

    nc.compile()
    return nc


def _get_program():
    global _prog
    if _prog is None:
        _prog = _build_program()
    return _prog


def _cent_ext(centroid):
    ce = np.zeros((K, 1088), dtype=np.float32)
    ce[:, :C] = centroid
    ce[:, C] = (centroid.astype(np.float64) ** 2).sum(1).astype(np.float32)
    return ce


def _host_prep(inputs):
    import ml_dtypes
    bfd = ml_dtypes.bfloat16
    g = lambda k: np.ascontiguousarray(np.asarray(inputs[k], dtype=np.float32))
    feature = g('feature')               # (B, C, T)
    centroid = g('centroid')             # (K, C)
    codebooks = g('codebooks')           # (NQ, BINS, D)

    feats = np.zeros((B, C, TPAD), dtype=np.float32)
    feats[:, :, :T] = feature

    base64 = np.broadcast_to(
        (1024.0 * (np.arange(64) // 8)).astype(np.float32), (128, 64)).copy()
    binbase = (np.arange(128)[:, None] +
               128.0 * np.arange(8)[None, :]).astype(np.float32)
    # cbmm[p, q, c, dc, d2] = cb[q, c*128+p, dc*128+d2]
    cbmm = np.ascontiguousarray(
        codebooks.reshape(NQ, 8, 128, NDC, 128).transpose(2, 0, 1, 3, 4)
    ).astype(bfd)

    shared = {
        "centbf": np.ascontiguousarray((2.0 * centroid.T)).astype(bfd),
        "cnc": np.ascontiguousarray(np.broadcast_to(
            (CSHIFT - (centroid.astype(np.float64) ** 2).sum(1)
             ).astype(np.float32)[None, :], (128, K))),
        "cent": centroid,
        "cent_ext": _cent_ext(centroid),
        "base64": base64,
        "binbase": binbase,
        "cbT2bf": np.ascontiguousarray(
            2.0 * codebooks.transpose(0, 2, 1)).astype(bfd),
        # tiny per-bin epsilon makes the fp32 row-max unique, so the
        # is_equal one-hot never selects two bins at once
        "cbnc": ((CBSHIFT - (codebooks.astype(np.float64) ** 2).sum(-1))
                 + np.arange(BINS)[None, :] * 2.0 ** -14
                 ).astype(np.float32),
        "cbmm": cbmm,
    }
    for pre in ("enc", "nrm", "dec"):
        for i in range(3):
            shared[f"{pre}_w{i}"] = g(f"{pre}_w{i}").astype(bfd)
            shared[f"{pre}_b{i}"] = g(f"{pre}_b{i}")

    in_maps = []
    for b in range(B):
        m = dict(shared)
        m["feat"] = np.ascontiguousarray(feats[b])
        m["featbf"] = np.ascontiguousarray(feats[b]).astype(bfd)
        in_maps.append(m)
    return in_maps


def kernel(**inputs):
    global LAST_RESULT
    from concourse.bass_utils import run_bass_kernel_spmd
    nc = _get_program()
    in_maps = _host_prep(inputs)
    kwargs = {}
    if TRACE:
        try:
            from ntff_shim import install_ntff_hook
            install_ntff_hook()
            kwargs["trace"] = True
        except Exception:
            pass
    res = run_bass_kernel_spmd(nc, in_maps, core_ids=list(range(B)), **kwargs)
    LAST_RESULT = res
    out = np.empty((B, C, T), dtype=np.float32)
    for b in range(B):
        out[b] = res.results[b]["out"]
    return out
